# revision 15
# baseline (speedup 1.0000x reference)
"""Nested-dropout (prefix) masking kernel for Trainium2.

out[n, j] = input[n, j] if j < b[n] else 0

Full shapes: input [16384, 4096] f32, b [16384] int (1 <= b <= 4096).
Sharding: pure data parallel over the batch dim across 8 cores
(2048 rows per core).

Per-core device kernel (SPMD, one Bass program):
  - iota[128, 4096] generated once on gpsimd, cast to f32
  - per 128-row tile: DMA in, out = (iota < b[p]) * x via one fused
    scalar_tensor_tensor op on the DVE, DMA out
"""

import numpy as np

import concourse.bacc as bacc
import concourse.bass as bass
import concourse.mybir as mybir
from concourse.bass_utils import run_bass_kernel_spmd
from concourse.tile import TileContext

N_CORES = 8
BATCH = 16384
K = 4096
P = 128  # partitions
ROWS_PER_CORE = BATCH // N_CORES  # 2048
TILES = ROWS_PER_CORE // P  # 16


def _build_bass():
    nc = bacc.Bacc("TRN2", target_bir_lowering=False)
    x = nc.dram_tensor("x", [ROWS_PER_CORE, K], mybir.dt.float32, kind="ExternalInput")
    b2d = nc.dram_tensor("b2d", [P, TILES], mybir.dt.float32, kind="ExternalInput")
    out = nc.dram_tensor(
        "out", [ROWS_PER_CORE, K], mybir.dt.float32, kind="ExternalOutput"
    )

    with TileContext(nc) as tc:
        with (
            tc.tile_pool(name="const", bufs=1) as const_pool,
            tc.tile_pool(name="xp", bufs=4) as xp,
            tc.tile_pool(name="mp", bufs=2) as mp,
        ):
            iota_i = const_pool.tile([P, K], mybir.dt.int32)
            nc.gpsimd.iota(iota_i[:], pattern=[[1, K]], channel_multiplier=0)
            iota_f = const_pool.tile([P, K], mybir.dt.float32)
            nc.vector.tensor_copy(out=iota_f[:], in_=iota_i[:])

            b_sb = const_pool.tile([P, TILES], mybir.dt.float32)
            nc.sync.dma_start(out=b_sb[:], in_=b2d[:])

            for t in range(TILES):
                xt = xp.tile([P, K], mybir.dt.float32)
                nc.sync.dma_start(out=xt[:], in_=x[t * P : (t + 1) * P, :])
                mt = mp.tile([P, K], mybir.dt.float32)
                # mask = (iota < b), then xt *= mask (in place)
                nc.vector.tensor_scalar(
                    out=mt[:],
                    in0=iota_f[:],
                    scalar1=b_sb[:, t : t + 1],
                    scalar2=None,
                    op0=mybir.AluOpType.is_lt,
                )
                nc.vector.tensor_tensor(
                    out=xt[:], in0=xt[:], in1=mt[:], op=mybir.AluOpType.mult
                )
                nc.sync.dma_start(out=out[t * P : (t + 1) * P, :], in_=xt[:])
    nc.compile()
    return nc


def _prepare(x, bv):
    nc = _build_bass()
    in_maps = []
    for c in range(N_CORES):
        sl = slice(c * ROWS_PER_CORE, (c + 1) * ROWS_PER_CORE)
        b_shard = bv[sl].astype(np.float32)
        # b2d[p, t] = b of row t*128 + p within the shard
        b2d = np.ascontiguousarray(b_shard.reshape(TILES, P).T)
        in_maps.append({"x": x[sl], "b2d": b2d})
    return nc, in_maps


def _run(input, b, trace=False):
    x = np.ascontiguousarray(np.asarray(input, dtype=np.float32))
    bv = np.asarray(b).astype(np.int32)
    assert x.shape == (BATCH, K), x.shape
    assert bv.shape == (BATCH,), bv.shape

    nc, in_maps = _prepare(x, bv)

    res = run_bass_kernel_spmd(
        nc,
        in_maps,
        core_ids=list(range(N_CORES)),
        trace=trace,
    )
    out = np.concatenate([r["out"] for r in res.results], axis=0)
    return out, res


def kernel(input, b):
    out, _ = _run(input, b)
    return out


# revision 21
# speedup vs baseline: 68.0251x; 68.0251x over previous
"""Nested-dropout (prefix) masking kernel for Trainium2.

out[n, j] = input[n, j] if j < b[n] else 0

Full shapes: input [16384, 4096] f32, b [16384] int (1 <= b <= 4096).
Sharding: pure data parallel over the batch dim across 8 cores
(2048 rows per core).

Strategy: b is known on the host when the BIR is built, so each core's
rows are processed in descending-b order (via indirect-DMA gather /
scatter with host-computed element offsets) and every 128-row tile only
reads and writes the first W_t = roundup(max b in tile) columns.  The
output buffer is zero-initialized by the runner (both the native
run_neff path and the PJRT donation path guarantee this), so the
all-zero suffix of each row is never touched.  Expected traffic is
~E[b]/K = half of the naive kernel's.

The mask itself is applied on the DVE with a single fused
scalar_tensor_tensor op per tile, restricted to the [lo_t, W_t) column
band where rows of the tile actually cut off; columns below lo_t are
kept as-is.
"""

import numpy as np

import concourse.bacc as bacc
import concourse.bass as bass
import concourse.mybir as mybir
from concourse.bass_utils import run_bass_kernel_spmd
from concourse.tile import TileContext

N_CORES = 8
BATCH = 16384
K = 4096
P = 128  # partitions
ROWS_PER_CORE = BATCH // N_CORES  # 2048
TILES = ROWS_PER_CORE // P  # 16
ALIGN = 16  # width roundup, elements


def _plan(bv):
    """Host-side plan: per-core sorted row order + shared tile widths.

    Returns (offs, b2d, widths, lows):
      offs[c][p, t]  int32 element offset (row*K) of the row handled by
                     tile t, partition p on core c
      b2d[c][p, t]   float32 b of that row
      widths[t]      shared (max over cores) column count read/written
      lows[t]        shared (min over cores) start of the masked band
    """
    offs, b2d = [], []
    hi = np.zeros((N_CORES, TILES), np.int64)
    lo = np.zeros((N_CORES, TILES), np.int64)
    for c in range(N_CORES):
        bs = bv[c * ROWS_PER_CORE : (c + 1) * ROWS_PER_CORE].astype(np.int64)
        order = np.argsort(-bs, kind="stable").astype(np.int64)
        bsort = bs[order]
        hi[c] = bsort.reshape(TILES, P).max(axis=1)
        lo[c] = bsort.reshape(TILES, P).min(axis=1)
        offs.append(
            np.ascontiguousarray(order.reshape(TILES, P).T.astype(np.int32))
        )
        b2d.append(np.ascontiguousarray(bsort.reshape(TILES, P).T.astype(np.float32)))
    widths = np.minimum(
        ((hi.max(axis=0) + ALIGN - 1) // ALIGN) * ALIGN, K
    ).astype(int)
    widths = np.maximum(widths, ALIGN)
    lows = lo.min(axis=0).astype(int)
    return offs, b2d, widths, lows


def _build_bass(widths, lows, reps=1):
    nc = bacc.Bacc("TRN2", target_bir_lowering=False)
    x = nc.dram_tensor(
        "x", [ROWS_PER_CORE * K], mybir.dt.float32, kind="ExternalInput"
    )
    b2d = nc.dram_tensor("b2d", [P, TILES], mybir.dt.float32, kind="ExternalInput")
    offs = nc.dram_tensor("offs", [P, TILES], mybir.dt.int32, kind="ExternalInput")
    out = nc.dram_tensor(
        "out", [ROWS_PER_CORE * K], mybir.dt.float32, kind="ExternalOutput"
    )
    # Row-granular indirect views: one DMA descriptor per row (the [N, 1]
    # element-granular form costs one descriptor per element).
    x2 = x[:].rearrange("(a b) -> a b", b=K)
    out2 = out[:].rearrange("(a b) -> a b", b=K)

    with TileContext(nc) as tc:
        with (
            tc.tile_pool(name="const", bufs=1) as const_pool,
            tc.tile_pool(name="xp", bufs=4) as xp,
        ):
            iota_i = const_pool.tile([P, K], mybir.dt.int32)
            nc.gpsimd.iota(iota_i[:], pattern=[[1, K]], channel_multiplier=0)
            iota_f = const_pool.tile([P, K], mybir.dt.float32)
            nc.vector.tensor_copy(out=iota_f[:], in_=iota_i[:])

            b_sb = const_pool.tile([P, TILES], mybir.dt.float32)
            nc.sync.dma_start(out=b_sb[:], in_=b2d[:])
            offs_sb = const_pool.tile([P, TILES], mybir.dt.int32)
            nc.sync.dma_start(out=offs_sb[:], in_=offs[:])

            for t in range(TILES * reps):
                t = t % TILES
                w = int(widths[t])
                lo = int(lows[t])
                xt = xp.tile([P, K], mybir.dt.float32)
                nc.gpsimd.indirect_dma_start(
                    out=xt[:, :w],
                    out_offset=None,
                    in_=x2,
                    in_offset=bass.IndirectOffsetOnAxis(
                        ap=offs_sb[:, t : t + 1], axis=0
                    ),
                )
                if lo < w:
                    # band [lo, w): xt = (iota < b) * xt, fused on the DVE
                    nc.vector.scalar_tensor_tensor(
                        out=xt[:, lo:w],
                        in0=iota_f[:, lo:w],
                        scalar=b_sb[:, t : t + 1],
                        in1=xt[:, lo:w],
                        op0=mybir.AluOpType.is_lt,
                        op1=mybir.AluOpType.mult,
                    )
                nc.gpsimd.indirect_dma_start(
                    out=out2,
                    out_offset=bass.IndirectOffsetOnAxis(
                        ap=offs_sb[:, t : t + 1], axis=0
                    ),
                    in_=xt[:, :w],
                    in_offset=None,
                )
    nc.compile()
    return nc


def _prepare(x, bv, reps=1):
    offs, b2d, widths, lows = _plan(bv)
    nc = _build_bass(widths, lows, reps=reps)
    in_maps = []
    for c in range(N_CORES):
        sl = slice(c * ROWS_PER_CORE, (c + 1) * ROWS_PER_CORE)
        in_maps.append(
            {"x": x[sl].reshape(-1), "b2d": b2d[c], "offs": offs[c]}
        )
    return nc, in_maps


def _run(input, b, trace=False):
    x = np.ascontiguousarray(np.asarray(input, dtype=np.float32))
    bv = np.asarray(b).astype(np.int64)
    assert x.shape == (BATCH, K), x.shape
    assert bv.shape == (BATCH,), bv.shape

    nc, in_maps = _prepare(x, bv)

    res = run_bass_kernel_spmd(
        nc,
        in_maps,
        core_ids=list(range(N_CORES)),
        trace=trace,
    )
    out = np.concatenate(
        [r["out"].reshape(ROWS_PER_CORE, K) for r in res.results], axis=0
    )
    return out, res


def kernel(input, b):
    out, _ = _run(input, b)
    return out
